# revision 58
# baseline (speedup 1.0000x reference)
"""CollaborativeAttention Trainium2 kernel.

Reference computation (B=16, S=512, D=512, H=8, DK=DV=DO=512, TB=64):
    q = x @ Wq.T ; k = x @ Wk.T
    mixed_q[b,h,s,i] = q[b,s,i] * mixing[h,i]
    scores = mixed_q @ k.T + tbias(T)[:,None] + cb.T[:, :, None, :]
    scores = mask(scores) / sqrt(64); probs = softmax(scores)
    v = (x @ Wv.T + bv) split into 8 heads of 64
    ctx = probs @ v ; out = ctx @ Wd.T + bd ; y = LayerNorm(x + out)

Key algebraic simplifications used here:
  * The 1->TB->1 leaky-relu MLP on u = 1/log(e+T) is piecewise-linear with all
    breakpoints at u = -b1/w1.  When no breakpoint falls inside the actual
    range of u (true for these inputs: b1 = 0, u > 0), the whole MLP collapses
    to  tbias = A*u + B  with scalars A, B computed from the weights.
  * softmax without max-subtraction (scores are O(1) for these inputs), with
    the additive mask turned into a multiplicative {0,1} mask applied to
    exp(tbias/8), so exp(scores) needs no -inf handling:
        probs ~ exp(s/8 + cb/8) * [exp(A'u + B') * M01]   (A'=A/8, B'=B/8)
  * normalization deferred past the probs@v matmul: ctx = (E@v) / (E@1).

Layout: everything is kept "T-major" ([t, s] / [d, s] with the contraction or
key index in partitions) so no on-device transposes are needed anywhere:
    qT[i,s], kT[i,t]       = Wq/Wk rows x xT
    scoresT[t,s]           = kT-slices.T @ (qT * mixing[h])
    PT[t,s]                = exp(scoresT + cb) * Eb
    ctxT[e,s], sums[s]     = v/ones.T @ PT
    out[s,o]               = ctxT-pairs.T @ WdT,   then residual + LayerNorm.

Sharding: pure data-parallel over batch, 2 batches per core, 8 cores, no
collectives; weights replicated.
"""

import math

import numpy as np

import ml_dtypes

import concourse.bass as bass
import concourse.mybir as mybir
import concourse.tile as tile
from concourse.bass_utils import run_bass_kernel_spmd

# ------------------------------------------------------------------ constants
B, S, D = 16, 512, 512
H = 8
DK = DV = DO = 512
TB = 64
EH = DV // H  # 64, per-head value dim
N_CORES = 8
BPC = B // N_CORES  # batches per core
KB = D // 128  # 4 k-blocks of 128
LN_EPS = 1e-5

F32 = mybir.dt.float32
BF16 = mybir.dt.bfloat16
F32R = mybir.dt.float32r

# mm dtype config: 'bf16' (default), 'f32r', 'f32'
CFG = {"mm": "bf16", "pt_engine": "vector"}


def _mm_np_dtype():
    return ml_dtypes.bfloat16 if CFG["mm"] == "bf16" else np.float32


def _mm_dt():
    return BF16 if CFG["mm"] == "bf16" else F32


def _mm_cast(ap):
    """View an fp32 AP as float32r for fast fp32 matmuls."""
    if CFG["mm"] == "f32r":
        return ap.bitcast(F32R)
    return ap


# ---------------------------------------------------------------- wait fixup
def _split_multi_waits(nc):
    """This walrus build allows 1 sync wait per instruction (2 on
    EventSemaphore).  Tile's final drain carries one wait per live semaphore;
    split the excess into preceding EventSemaphore instructions."""
    counter = 0
    for fn in nc.m.functions:
        for bb in fn.blocks:
            insts = bb.instructions
            i = 0
            while i < len(insts):
                inst = insts[i]
                si = inst.sync_info
                waits = list(si.on_wait) if si is not None else []
                cap = 2 if isinstance(inst, mybir.InstEventSemaphore) else 1
                if len(waits) > cap:
                    extra, keep = waits[:-cap], waits[-cap:]
                    new_evs = []
                    for j in range(0, len(extra), 2):
                        counter += 1
                        ev = mybir.InstEventSemaphore(
                            name=f"I-waitfix-{counter}",
                            engine=inst.engine,
                            ins=[],
                            outs=[],
                            sync_info=mybir.SyncInfo(
                                on_wait=extra[j : j + 2], on_update=[]
                            ),
                        )
                        nc.register_instruction(ev)
                        new_evs.append(ev)
                    inst.sync_info = mybir.SyncInfo(
                        on_wait=keep, on_update=list(si.on_update)
                    )
                    for k, ev in enumerate(new_evs):
                        insts.insert(i + k, ev)
                    i += len(new_evs)
                i += 1


# ---------------------------------------------------------------- host prep
def _tb_affine(tb1_w, tb1_b, tb2_w, tb2_b, u_min, u_max):
    """Collapse the temporal-bias MLP to tbias = A*u + B over u in
    [u_min, u_max].  Returns (A, B) or None if any leaky-relu breakpoint falls
    strictly inside the range (then the affine form is invalid)."""
    w1 = np.asarray(tb1_w, np.float64).reshape(-1)  # [TB]
    b1 = np.asarray(tb1_b, np.float64).reshape(-1)  # [TB]
    w2 = np.asarray(tb2_w, np.float64).reshape(-1)  # [TB]
    b2 = float(np.asarray(tb2_b, np.float64).reshape(-1)[0])
    lo = w1 * u_min + b1
    hi = w1 * u_max + b1
    if np.any((lo < 0) & (hi > 0)) or np.any((lo > 0) & (hi < 0)):
        return None
    pos = (lo + hi) > 0  # sign of the argument over the whole range
    f = np.where(pos, 1.0, 0.2)
    A = float(np.sum(w2 * f * w1))
    Bc = float(np.sum(w2 * f * b1) + b2)
    return A, Bc


def _prepare(inputs):
    x = np.asarray(inputs["x"], np.float32)
    T = np.asarray(inputs["batch_temporal_mat"], np.float32)
    Wq = np.asarray(inputs["Wq"], np.float32)
    Wk = np.asarray(inputs["Wk"], np.float32)
    Wcb = np.asarray(inputs["Wcb"], np.float32)
    Wv = np.asarray(inputs["Wv"], np.float32)
    bv = np.asarray(inputs["bv"], np.float32)
    mixing = np.asarray(inputs["mixing"], np.float32)
    Wd = np.asarray(inputs["Wd"], np.float32)
    bd = np.asarray(inputs["bd"], np.float32)
    ln_g = np.asarray(inputs["ln_g"], np.float32)
    ln_b = np.asarray(inputs["ln_b"], np.float32)

    inv_sqrt_hs = 1.0 / math.sqrt(DK / H)  # 1/8

    mmdt = _mm_np_dtype()

    # u = 1/log(e+T) range over the actual inputs
    L = np.log(np.e + T.astype(np.float64))
    u_min, u_max = float((1.0 / L).min()), float((1.0 / L).max())
    ab = _tb_affine(
        inputs["tb1_w"], inputs["tb1_b"], inputs["tb2_w"], inputs["tb2_b"],
        u_min, u_max,
    )

    # multiplicative causal mask in [t, s] layout; row0 (s=0) fully visible.
    # The s axis is rotated (s=0 moved to the end) so that for key-block t the
    # needed query columns [128t, 512) + {s=0} become one contiguous range --
    # scores/exp/PT/PV then run on right-aligned column slices only.
    PERM = np.concatenate([np.arange(1, S), [0]])
    t_idx = np.arange(S)[:, None]
    s_idx = np.arange(S)[None, :]
    m01 = np.where((t_idx > s_idx) & (s_idx != 0), 0.0, 1.0)  # [t, s]
    m01 = m01[:, PERM]

    flags = {
        "tb_affine": ab is not None,
        "bv_zero": not np.any(bv),
        "bd_zero": not np.any(bd),
        "ln_identity": bool(np.all(ln_g == 1.0) and not np.any(ln_b)),
    }

    if ab is not None:
        A, Bc = ab
        a_scaled = float(A * inv_sqrt_hs)
        mask_exp = (m01 * np.exp(Bc * inv_sqrt_hs)).astype(np.float32)
        tt_full = np.ascontiguousarray(T.transpose(0, 2, 1)[:, :, PERM])
        flags["a_scaled"] = a_scaled
    else:
        # generic fallback: compute exp(tbias/8)*mask on the host
        u = (1.0 / L).astype(np.float64)[..., None]  # [B,S,S,1]
        h = u * np.asarray(inputs["tb1_w"], np.float64).reshape(-1) + np.asarray(
            inputs["tb1_b"], np.float64
        ).reshape(-1)
        h = np.where(h > 0, h, 0.2 * h)
        tbias = h @ np.asarray(inputs["tb2_w"], np.float64).reshape(-1) + float(
            np.asarray(inputs["tb2_b"], np.float64).reshape(-1)[0]
        )  # [B,S,S] in [s,t]
        eb = np.exp(tbias * inv_sqrt_hs).transpose(0, 2, 1)[:, :, PERM] * m01
        tt_full = np.ascontiguousarray(eb).astype(np.float32)
        mask_exp = m01.astype(np.float32)  # unused on device, keep shape
        flags["a_scaled"] = 0.0

    xT = np.ascontiguousarray(x.transpose(0, 2, 1))  # [B, d, t] natural
    xTq = np.ascontiguousarray(xT[:, :, PERM])  # [B, d, s'] query-rotated

    common = {
        "wqt": np.ascontiguousarray(Wq.T).astype(mmdt),  # [d, i]
        "wkt": np.ascontiguousarray(Wk.T).astype(mmdt),  # [d, i]
        "wvt": np.ascontiguousarray(Wv.T).astype(mmdt),  # [d, j]
        "wdt": np.ascontiguousarray(Wd.T).astype(mmdt),  # [j, o]
        "wcbt": np.ascontiguousarray(Wcb.T * inv_sqrt_hs).astype(mmdt),  # [d, h]
        "mixt": np.ascontiguousarray(mixing.T * inv_sqrt_hs).astype(np.float32),
        "maskexp": mask_exp.astype(ml_dtypes.bfloat16),  # [t, s]
        "ones": np.ones((128, 512), mmdt),
        "consts": np.broadcast_to(
            np.array([np.e, LN_EPS], np.float32), (128, 2)
        ).copy(),
    }
    if not flags["bv_zero"]:
        common["bvrow"] = bv.reshape(1, DV).astype(mmdt)
    if not flags["bd_zero"]:
        common["bdrow"] = bd.reshape(1, DO).astype(mmdt)
    if not flags["ln_identity"]:
        common["lng"] = np.broadcast_to(ln_g, (128, DV)).astype(np.float32).copy()
        common["lnb"] = np.broadcast_to(ln_b, (128, DV)).astype(np.float32).copy()

    in_maps = []
    for c in range(N_CORES):
        sl = slice(c * BPC, (c + 1) * BPC)
        m = dict(common)
        m["xt"] = np.ascontiguousarray(xT[sl]).astype(mmdt)
        m["xtq"] = np.ascontiguousarray(xTq[sl]).astype(mmdt)
        m["xr"] = np.ascontiguousarray(x[sl][:, PERM, :])
        m["tt"] = np.ascontiguousarray(tt_full[sl])
        in_maps.append(m)
    return in_maps, flags


# -------------------------------------------------------------- device build
def build_nc(flags):
    mmdt = _mm_dt()
    nc = bass.Bass()

    xt_d = nc.dram_tensor("xt", [BPC, D, S], mmdt, kind="ExternalInput")
    xtq_d = nc.dram_tensor("xtq", [BPC, D, S], mmdt, kind="ExternalInput")
    xr_d = nc.dram_tensor("xr", [BPC, S, D], F32, kind="ExternalInput")
    tt_d = nc.dram_tensor("tt", [BPC, S, S], F32, kind="ExternalInput")
    wqt_d = nc.dram_tensor("wqt", [D, DK], mmdt, kind="ExternalInput")
    wkt_d = nc.dram_tensor("wkt", [D, DK], mmdt, kind="ExternalInput")
    wvt_d = nc.dram_tensor("wvt", [D, DV], mmdt, kind="ExternalInput")
    wdt_d = nc.dram_tensor("wdt", [DV, DO], mmdt, kind="ExternalInput")
    wcbt_d = nc.dram_tensor("wcbt", [D, H], mmdt, kind="ExternalInput")
    mixt_d = nc.dram_tensor("mixt", [DK, H], F32, kind="ExternalInput")
    maskexp_d = nc.dram_tensor("maskexp", [S, S], BF16, kind="ExternalInput")
    ones_d = nc.dram_tensor("ones", [128, 512], mmdt, kind="ExternalInput")
    consts_d = nc.dram_tensor("consts", [128, 2], F32, kind="ExternalInput")
    if not flags["bv_zero"]:
        bvrow_d = nc.dram_tensor("bvrow", [1, DV], mmdt, kind="ExternalInput")
    if not flags["bd_zero"]:
        bdrow_d = nc.dram_tensor("bdrow", [1, DO], mmdt, kind="ExternalInput")
    if not flags["ln_identity"]:
        lng_d = nc.dram_tensor("lng", [128, DV], F32, kind="ExternalInput")
        lnb_d = nc.dram_tensor("lnb", [128, DV], F32, kind="ExternalInput")
    y_d = nc.dram_tensor("y", [BPC, S, DO], F32, kind="ExternalOutput")

    a_scaled = flags.get("a_scaled", 0.0)
    tb_affine = flags["tb_affine"]

    mul = mybir.AluOpType.mult
    sub = mybir.AluOpType.subtract
    add = mybir.AluOpType.add
    AF = mybir.ActivationFunctionType

    from contextlib import ExitStack

    with tile.TileContext(nc) as tc:
        with ExitStack() as est:
            pool = lambda name, bufs, **kw: est.enter_context(
                tc.tile_pool(name=name, bufs=bufs, **kw)
            )
            big = CFG["mm"] != "bf16"  # fp32-sized tiles: shrink buffering
            wts = pool("wts", 1)
            xt_p = pool("xt", 1 if big else 2)
            xr_p = pool("xr", 1 if big else 2)
            tt_p = pool("tt", 1 if big else 2)
            ebw_p = pool("ebw", 2)
            eb_p = pool("eb", 1 if big else 2)
            qkv_p = pool("qkv", 1 if big else 2)
            qt_p = pool("qtp", 5 if big else 8)
            vp_p = pool("vp", 4)
            cb_p = pool("cb", 2)
            mq_p = pool("mq", 5 if big else 10)
            ptx_p = pool("ptx", 4)
            pt_p = pool("pt", 5 if big else 12)
            rs_p = pool("rs", 2)
            ctx_p = pool("ctx", 4 if big else 8)
            ysb_p = pool("ysb", 3)
            scr_p = pool("scr", 2)
            yout_p = pool("yout", 4)
            st_p = pool("st", 24)
            psA = pool("psA", 2, space="PSUM")
            psD = pool("psD", 2, space="PSUM")
            psS = pool("psS", 2, space="PSUM")
            psCU = pool("psCU", 2, space="PSUM")
            # ---- resident weights.  DMA emission order controls HWDGE queue
            # assignment (round-robin), so first-needed data is issued first
            # and large transfers are split across queues.
            def dma_split_kp(dst, src3):
                # per-k-block chunks, each split into partition halves: twice
                # the queue parallelism at identical descriptor efficiency
                for c in range(KB):
                    for ph in range(2):
                        nc.sync.dma_start(
                            dst[ph * 64 : (ph + 1) * 64, c, :],
                            src3[c * 128 + ph * 64 : c * 128 + (ph + 1) * 64, :],
                        )

            def dma_split_k(dst, src3, nchunks=KB, shalves=1):
                # src3: [D, N] dram; dst: [128, KB, N] tile; chunk = k-block
                # (optionally split each chunk's free dim into halves too)
                per = KB // nchunks
                n = src3.shape[1]
                hw_ = n // shalves
                for c in range(nchunks):
                    for sh in range(shalves):
                        nc.sync.dma_start(
                            dst[
                                :,
                                c * per : (c + 1) * per,
                                sh * hw_ : (sh + 1) * hw_,
                            ],
                            src3[
                                c * per * 128 : (c + 1) * per * 128,
                                sh * hw_ : (sh + 1) * hw_,
                            ].rearrange("(k p) n -> p k n", p=128),
                        )

            # HAM warmup: the PE would otherwise idle ~13us waiting for the
            # first DMAs and start the real matmuls at the throttled 1.2 GHz
            # clock.  Junk matmuls on a memset tile keep the activity monitor
            # busy through the DMA window so real work starts at 2.4 GHz.
            warm = wts.tile([128, 512], mmdt, tag="warm")
            nc.gpsimd.memset(warm[:], 0.0)
            for _wi in range(30):
                wp = psS.tile([128, 512], F32, tag="psS", name="warm_ps")
                nc.tensor.matmul(
                    wp[:],
                    _mm_cast(warm[:, 0:128]),
                    _mm_cast(warm[:]),
                    start=True,
                    stop=True,
                )

            wqt = wts.tile([128, KB, DK], mmdt, tag="wqt")
            dma_split_k(wqt, wqt_d[:], KB)
            wkt = wts.tile([128, KB, DK], mmdt, tag="wkt")
            dma_split_k(wkt, wkt_d[:], KB)
            wvt = wts.tile([128, KB, DV], mmdt, tag="wvt")
            wdt = wts.tile([128, KB, DO], mmdt, tag="wdt")
            wcbt = wts.tile([128, KB, H], mmdt, tag="wcbt")
            mixt = wts.tile([128, KB, H], F32, tag="mixt")
            maskexp = [
                wts.tile([128, S], BF16, tag=f"maskexp{t}", name="maskexp")
                for t in range(KB)
            ]
            ones_sb = wts.tile([128, 512], mmdt, tag="ones")
            consts = wts.tile([128, 2], F32, tag="consts")
            if not flags["bv_zero"]:
                bvrow = wts.tile([1, DV], mmdt, tag="bvrow")
            if not flags["bd_zero"]:
                bdrow = wts.tile([1, DO], mmdt, tag="bdrow")
            if not flags["ln_identity"]:
                lng = wts.tile([128, DV], F32, tag="lng")
                lnb = wts.tile([128, DV], F32, tag="lnb")

            def load_secondary_weights():
                dma_split_k(wvt, wvt_d[:], 2)
                nc.sync.dma_start(
                    wcbt[:], wcbt_d[:].rearrange("(k p) h -> p k h", p=128)
                )
                nc.sync.dma_start(
                    mixt[:], mixt_d[:].rearrange("(k p) h -> p k h", p=128)
                )
                for t in range(KB):
                    nc.sync.dma_start(
                        maskexp[t][:], maskexp_d[bass.ts(t, 128), :]
                    )
                dma_split_k(wdt, wdt_d[:], 2)
                nc.sync.dma_start(ones_sb[:], ones_d[:])
                nc.sync.dma_start(consts[:], consts_d[:])
                if not flags["bv_zero"]:
                    nc.sync.dma_start(bvrow[:], bvrow_d[:])
                if not flags["bd_zero"]:
                    nc.sync.dma_start(bdrow[:], bdrow_d[:])
                if not flags["ln_identity"]:
                    nc.sync.dma_start(lng[:], lng_d[:])
                    nc.sync.dma_start(lnb[:], lnb_d[:])

            def emit_stage_c(bb, ctxs, xr):
                    # ---- dense + residual + layernorm (for batch bb)
                    for sb in range(KB):
                        dps = psD.tile([128, 512], F32, tag="psD")
                        for p in range(KB):
                            last = p == KB - 1 and flags["bd_zero"]
                            nc.tensor.matmul(
                                dps[:],
                                _mm_cast(ctxs[p][:, bass.ts(sb, 128)]),
                                _mm_cast(wdt[:, p, :]),
                                start=(p == 0),
                                stop=last,
                            )
                        if not flags["bd_zero"]:
                            nc.tensor.matmul(
                                dps[:],
                                _mm_cast(ones_sb[0:1, 0:128]),
                                _mm_cast(bdrow[:]),
                                start=False,
                                stop=True,
                            )
                        ysb = ysb_p.tile([128, DO], F32, tag="ysb")
                        rowsum = st_p.tile([128, 1], F32, tag="st")
                        nc.vector.scalar_tensor_tensor(
                            out=ysb[:],
                            in0=dps[:],
                            scalar=0.0,
                            in1=xr[:, sb, :],
                            op0=add,
                            op1=add,
                            accum_out=rowsum[:],
                        )
                        scr = scr_p.tile([128, DO], F32, tag="scr")
                        rsumsq = st_p.tile([128, 1], F32, tag="st")
                        nc.vector.scalar_tensor_tensor(
                            out=scr[:],
                            in0=ysb[:],
                            scalar=1.0,
                            in1=ysb[:],
                            op0=mul,
                            op1=mul,
                            accum_out=rsumsq[:],
                        )
                        mu = st_p.tile([128, 1], F32, tag="st")
                        nc.vector.tensor_scalar_mul(mu[:], rowsum[:], 1.0 / DO)
                        var = st_p.tile([128, 1], F32, tag="st")
                        nc.vector.tensor_scalar(
                            out=var[:],
                            in0=rsumsq[:],
                            scalar1=1.0 / DO,
                            scalar2=None,
                            op0=mul,
                        )
                        musq = st_p.tile([128, 1], F32, tag="st")
                        nc.vector.tensor_scalar(
                            out=musq[:], in0=mu[:], scalar1=mu[:], scalar2=None, op0=mul
                        )
                        sd = st_p.tile([128, 1], F32, tag="st")
                        # var = E[y^2] - mu^2 ; rstd = exp(-0.5*ln(var+eps))
                        nc.vector.tensor_scalar(
                            out=sd[:], in0=var[:], scalar1=musq[:], scalar2=None, op0=sub
                        )
                        lnv = st_p.tile([128, 1], F32, tag="st")
                        nc.scalar.activation(lnv[:], sd[:], AF.Ln, bias=consts[:, 1:2])
                        rstd = st_p.tile([128, 1], F32, tag="st")
                        nc.scalar.activation(rstd[:], lnv[:], AF.Exp, scale=-0.5)
                        m2 = st_p.tile([128, 1], F32, tag="st")
                        nc.vector.tensor_scalar(
                            out=m2[:], in0=mu[:], scalar1=rstd[:], scalar2=None, op0=mul
                        )
                        zdst = yout_p.tile([128, DO], F32, tag="yz")
                        nc.vector.tensor_scalar(
                            out=zdst[:],
                            in0=ysb[:],
                            scalar1=rstd[:],
                            scalar2=m2[:],
                            op0=mul,
                            op1=sub,
                        )
                        if not flags["ln_identity"]:
                            z2 = ysb_p.tile([128, DO], F32, tag="z2")
                            nc.vector.tensor_mul(z2[:], zdst[:], lng[:])
                            zf = yout_p.tile([128, DO], F32, tag="yzf")
                            nc.vector.tensor_add(zf[:], z2[:], lnb[:])
                            zdst = zf
                        nc.sync.dma_start(y_d[bb, bass.ts(sb, 128), :], zdst[:])


            pending = []
            for b in range(BPC):
                # ---- load per-batch activations
                xtq = xt_p.tile([128, KB, S], mmdt, tag="xtq")
                dma_split_k(xtq, xtq_d[b], KB)
                xt = xt_p.tile([128, KB, S], mmdt, tag="xt")
                dma_split_k(xt, xt_d[b], KB)
                tt = tt_p.tile([128, KB, S], F32, tag="tt")
                dma_split_k(tt, tt_d[b])
                if b == 0:
                    load_secondary_weights()

                # residual input is only needed in the output stage; issue its
                # DMA after the early loads so it doesn't steal queue slots
                xr = xr_p.tile([128, KB, D], F32, tag="xr")
                dma_split_k(xr, xr_d[b])

                # ---- qT, kT, v projections ([i/j in partitions, s/t free])
                qt = [
                    qt_p.tile([128, DK], mmdt, tag="qt", name="qt")
                    for _ in range(KB)
                ]
                kt = qkv_p.tile([128, KB, DK], mmdt, tag="kt")
                vt = qkv_p.tile([128, KB, DV], mmdt, tag="vt")
                # ---- Eb = exp(tbias/8) * mask  (bf16, [t, s] layout),
                # emitted in per-t-block chunks interleaved with the q/k
                # copies so ACT produces eb[t] early for the first heads
                eb = [
                    eb_p.tile([128, S], BF16, tag=f"eb{t}", name="eb")
                    for t in range(KB)
                ]

                def emit_eb_chunk(t):
                    a = 0 if t == 0 else (t * 128 - 2) // 32 * 32
                    if tb_affine:
                        w0 = ebw_p.tile([128, S], F32, tag="ebw", name="ebw")
                        nc.scalar.activation(
                            w0[:, a:], tt[:, t, a:], AF.Ln, bias=consts[:, 0:1]
                        )
                        w1 = ebw_p.tile([128, S], F32, tag="ebw", name="ebw")
                        nc.scalar.activation(w1[:, a:], w0[:, a:], AF.Ln)
                        w2 = ebw_p.tile([128, S], F32, tag="ebw", name="ebw")
                        nc.scalar.activation(w2[:, a:], w1[:, a:], AF.Exp, scale=-1.0)
                        w3 = ebw_p.tile([128, S], F32, tag="ebw", name="ebw")
                        nc.scalar.activation(
                            w3[:, a:], w2[:, a:], AF.Exp, scale=a_scaled
                        )
                        nc.gpsimd.tensor_mul(
                            eb[t][:, a:], w3[:, a:], maskexp[t][:, a:]
                        )
                    else:
                        nc.gpsimd.tensor_mul(
                            eb[t][:, a:], tt[:, t, a:], maskexp[t][:, a:]
                        )

                for wi, (w, src_t, dst2, dst3) in enumerate(
                    ((wqt, xtq, qt, None), (wkt, xt, None, kt))
                ):
                    # dst[i, s] = sum_d w[d, i] * xT[d, s]   (i in partitions)
                    for i in range(KB):
                        ps = psA.tile([128, 512], F32, tag="psA", name="ps")
                        for k in range(KB):
                            nc.tensor.matmul(
                                ps[:],
                                _mm_cast(w[:, k, bass.ts(i, 128)]),
                                _mm_cast(src_t[:, k, :]),
                                start=(k == 0),
                                stop=(k == KB - 1),
                            )
                        dst = dst2[i][:] if dst2 is not None else dst3[:, i, :]
                        nc.vector.tensor_copy(dst, ps[:])
                        if i % 2 == 1:
                            emit_eb_chunk(wi * 2 + i // 2)
                # v[t, j] = sum_d xT[d, t] * Wv.T[d, j] (+ bv)  (t in partitions)
                for i in range(KB):
                    ps = psA.tile([128, 512], F32, tag="psA")
                    for k in range(KB):
                        last = k == KB - 1 and flags["bv_zero"]
                        nc.tensor.matmul(
                            ps[:],
                            _mm_cast(xt[:, k, bass.ts(i, 128)]),
                            _mm_cast(wvt[:, k, :]),
                            start=(k == 0),
                            stop=last,
                        )
                    if not flags["bv_zero"]:
                        nc.tensor.matmul(
                            ps[:],
                            _mm_cast(ones_sb[0:1, 0:128]),
                            _mm_cast(bvrow[:]),
                            start=False,
                            stop=True,
                        )
                    nc.vector.tensor_copy(vt[:, i, :], ps[:])

                # ---- content bias cb ([t in partitions, h free], f32, /8)
                cbps = psA.tile([128, 512], F32, tag="psA")
                for i in range(KB):
                    for k in range(KB):
                        nc.tensor.matmul(
                            cbps[:, bass.ts(i, H)],
                            _mm_cast(xt[:, k, bass.ts(i, 128)]),
                            _mm_cast(wcbt[:, k, :]),
                            start=(k == 0),
                            stop=(k == KB - 1),
                        )
                cb = cb_p.tile([128, 32], F32, tag="cb")
                nc.scalar.copy(cb[:], cbps[:, 0:32])

                # previous batch's output stage goes here so the PE has
                # dense work while this batch's first heads wait on DVE/ACT
                if pending:
                    emit_stage_c(*pending.pop())

                # ---- per-head attention
                ctxs = []
                for h in range(H):
                    mq = mq_p.tile([128, KB, S], _mm_dt(), tag="mq")
                    for k in range(KB):
                        nc.vector.tensor_scalar_mul(
                            mq[:, k, :], qt[k][:], mixt[:, k, h : h + 1]
                        )
                    mqs = lambda k: mq[:, k, :]
                    if h % 2 == 0:
                        ctxpk = ctx_p.tile([128, S], _mm_dt(), tag="ctx")
                    pts = []
                    for t in range(KB):
                        a = 0 if t == 0 else (t * 128 - 2) // 32 * 32
                        w_ = S - a
                        sps = psS.tile([128, S], F32, tag="psS", name="sps")
                        for k in range(KB):
                            nc.tensor.matmul(
                                sps[:, a:],
                                _mm_cast(kt[:, k, bass.ts(t, 128)]),
                                _mm_cast(mq[:, k, a:]),
                                start=(k == 0),
                                stop=(k == KB - 1),
                            )
                        ptx = ptx_p.tile([128, S], _mm_dt(), tag="ptx")
                        nc.scalar.activation(
                            ptx[:, a:], sps[:, a:], AF.Exp,
                            bias=cb[:, H * t + h : H * t + h + 1],
                        )
                        pt = pt_p.tile([128, S], _mm_dt(), tag="pt")
                        eng = CFG["pt_engine"]
                        if eng == "alt":
                            eng = "gpsimd" if t <= 1 else "vector"
                        e = nc.gpsimd if eng == "gpsimd" else nc.vector
                        e.tensor_mul(pt[:, a:], ptx[:, a:], eb[t][:, a:])
                        pts.append((pt, a))
                    # ctxU[e, s] (rows 0:64) and sums[s] (rows 64:128) packed
                    # into one PSUM bank via col-group tile_position
                    cu = psCU.tile([128, S], F32, tag="psCU")
                    cps = cu[0:64, :]
                    ups = cu[64:128, :]
                    for t in range(KB):
                        pt, a = pts[t]
                        nc.tensor.matmul(
                            cps[:, a:],
                            _mm_cast(vt[:, t, bass.ts(h, EH)]),
                            _mm_cast(pt[:, a:]),
                            start=(t == 0),
                            stop=(t == KB - 1),
                            tile_position=(0, 0),
                            skip_group_check=True,
                        )
                        nc.tensor.matmul(
                            ups[:, a:],
                            _mm_cast(ones_sb[:, 0:64]),
                            _mm_cast(pt[:, a:]),
                            start=(t == 0),
                            stop=(t == KB - 1),
                            tile_position=(0, 64),
                            skip_group_check=True,
                        )
                    rsln = rs_p.tile([64, S], F32, tag="rsln")
                    nc.scalar.activation(rsln[:], ups, AF.Ln)
                    rsum = rs_p.tile([64, S], F32, tag="rs")
                    nc.scalar.activation(rsum[:], rsln[:], AF.Exp, scale=-1.0)
                    nc.vector.tensor_mul(
                        ctxpk[bass.ts(h % 2, 64), :], cps, rsum[:]
                    )
                    if h % 2 == 1:
                        ctxs.append(ctxpk)
                if True:
                    pending.append((b, ctxs, xr))

            if pending:
                emit_stage_c(*pending.pop())

    _split_multi_waits(nc)
    return nc


# ------------------------------------------------------------------- driver
def _run(inputs, trace=False, trace_kwargs=None):
    in_maps, flags = _prepare(inputs)
    nc = build_nc(flags)
    res = run_bass_kernel_spmd(
        nc,
        in_maps,
        list(range(N_CORES)),
        trace=trace,
        **(trace_kwargs or {}),
    )
    PERM = np.concatenate([np.arange(1, S), [0]])
    out = np.empty((B, S, DO), np.float32)
    for c in range(N_CORES):
        out[c * BPC : (c + 1) * BPC][:, PERM, :] = res.results[c]["y"]
    return out, res


def kernel(**inputs) -> np.ndarray:
    out, _ = _run(inputs, trace=False)
    return out
